# revision 6
# baseline (speedup 1.0000x reference)
"""GNN edge-softmax attention kernel for 8 Trainium2 NeuronCores — v2.

Strategy (8-way src-tile partition, zero collectives, zero dma_gather):
  - Host packs nodes into 128-slot tiles balanced by edge count (392 tiles),
    assigns 49 tiles to each core (LPT), and routes every edge to the core
    owning its src tile.  Per-core tile order is sorted by edge count so the
    shared block layout (bpt = max over cores) wastes <3% padding.
  - The host pre-gathers RAW k[dst] / v[dst] rows per edge (pure data
    movement; all arithmetic stays on device) as f16, k-tile-split for
    accumulating matmuls.  Each core streams these sequentially - no
    indirect DMA, no dma_gather descriptor-generation bottleneck.
  - Per 128-edge block on device: PE projects kh_e / vh_e (f16 matmuls),
    one-hot-gathers qh rows (ST matmul), computes edge-feature logits (eb);
    DVE forms qk = sum_d qh*kh, adds eb, ACT exponentiates; DVE weights vh;
    PE scatter-accumulates num|den into PSUM via one-hot S.
  - Finalize runs inline per tile (normalize + output projection); each core
    writes only its own 49 tiles.  Host re-permutes rows to node ids.
"""

import math
import sys

import numpy as np

sys.path.insert(0, "/opt/trn_rl_repo")

import concourse.bacc as bacc
import concourse.bass as bass
import concourse.mybir as mybir
import concourse.tile as tile
from concourse import bass_utils

F16 = mybir.dt.float16
F8 = mybir.dt.float8e4
F32 = mybir.dt.float32

H = 8            # heads
D = 16           # head dim
TD = H * D       # 128
QD = 256         # q/k/v feature dim
PD = 64          # edge pair feature dim
NC = 8           # cores
P = 128

AF = mybir.ActivationFunctionType
ALU = mybir.AluOpType
DR = mybir.MatmulPerfMode.DoubleRow


def _f8(x):
    import ml_dtypes
    return np.asarray(x, np.float32).astype(ml_dtypes.float8_e4m3)


def prepare(q, k, v, edges, edge_index, Wq, Wk, Wv, Wb, bb, Wo, bo):
    N = q.shape[0]
    T = NC * math.ceil(N / (NC * P))             # global tiles, mult of NC
    TPC = T // NC                                # tiles per core

    src = np.asarray(edge_index[:, 0], dtype=np.int64)
    dst = np.asarray(edge_index[:, 1], dtype=np.int64)
    deg = np.bincount(src, minlength=N)

    # --- greedy node->tile packing balanced by edge count ---
    order = np.argsort(-deg, kind="stable")
    tile_cnt = np.zeros(T, dtype=np.int64)
    tile_edges = np.zeros(T, dtype=np.int64)
    node_tile = np.zeros(N, dtype=np.int32)
    node_slot = np.zeros(N, dtype=np.int32)
    import heapq
    heap = [(0, t) for t in range(T)]
    heapq.heapify(heap)
    for n in order:
        while True:
            e_cnt, t = heapq.heappop(heap)
            if tile_cnt[t] < P:
                break
        node_tile[n] = t
        node_slot[n] = tile_cnt[t]
        tile_cnt[t] += 1
        tile_edges[t] += deg[n]
        if tile_cnt[t] < P:
            heapq.heappush(heap, (tile_edges[t], t))

    # --- tile -> core (LPT) then per-core order by count desc ---
    t_order = np.argsort(-tile_edges, kind="stable")
    core_load = [(0, c) for c in range(NC)]
    heapq.heapify(core_load)
    core_tiles = [[] for _ in range(NC)]
    for t in t_order:
        load, c = heapq.heappop(core_load)
        core_tiles[c].append(t)
        heapq.heappush(core_load, (load + int(tile_edges[t]), c))
    # per-core local order: by edge count desc (aligns heavy tiles at the
    # same tloc across cores so the shared bpt is tight)
    for c in range(NC):
        core_tiles[c].sort(key=lambda t: -int(tile_edges[t]))
    tile_of = np.zeros((NC, TPC), dtype=np.int64)    # (core, tloc) -> tile
    core_of_tile = np.zeros(T, dtype=np.int64)
    tloc_of_tile = np.zeros(T, dtype=np.int64)
    for c in range(NC):
        for i, t in enumerate(core_tiles[c]):
            tile_of[c, i] = t
            core_of_tile[t] = c
            tloc_of_tile[t] = i

    cnt = np.zeros((NC, TPC), dtype=np.int64)
    for c in range(NC):
        cnt[c] = tile_edges[tile_of[c]]
    bpt = np.maximum(1, np.ceil(cnt.max(axis=0) / P).astype(np.int64))  # [TPC]
    blk_off = np.concatenate([[0], np.cumsum(bpt)])
    NBLK = int(blk_off[-1])
    ECAP = NBLK * P

    # --- per-core edge arrays ---
    import ml_dtypes
    F8NP = ml_dtypes.float8_e4m3
    E3NP = ml_dtypes.float8_e3m4
    k8 = np.asarray(k, np.float32).astype(np.float16)   # [N, QD]
    v8 = np.asarray(v, np.float32).astype(np.float16)
    e8 = np.asarray(edges, np.float32).astype(E3NP)     # [E, PD]

    edge_core = core_of_tile[node_tile[src]]
    edge_tloc = tloc_of_tile[node_tile[src]]

    cores = []
    for c in range(NC):
        mask = edge_core == c
        es, ed, et = src[mask], dst[mask], edge_tloc[mask]
        ordr = np.argsort(et, kind="stable")
        es, ed, et = es[ordr], ed[ordr], et[ordr]
        eidx = np.nonzero(mask)[0][ordr]
        # position per edge: tile t's edges fill from blk_off[t]*P
        pos = np.zeros(len(es), dtype=np.int64)
        start = 0
        for t in range(TPC):
            ccc = int(cnt[c, t])
            pos[start:start + ccc] = blk_off[t] * P + np.arange(ccc)
            start += ccc

        # per-edge raw k/v rows, k-tile-split layout [128, NBLK, 2, 128]
        kT = np.zeros((P, NBLK, 2, P), dtype=np.float16)
        vT = np.zeros((P, NBLK, 2, P), dtype=np.float16)
        kr = np.zeros((ECAP, QD), dtype=np.float16)
        vr = np.zeros((ECAP, QD), dtype=np.float16)
        kr[pos] = k8[ed]
        vr[pos] = v8[ed]
        # [ECAP, 256] -> [NBLK, 128e, 2, 128kp] -> [kp, b, s, e]
        kT[:] = kr.reshape(NBLK, P, 2, P).transpose(3, 0, 2, 1)
        vT[:] = vr.reshape(NBLK, P, 2, P).transpose(3, 0, 2, 1)
        del kr, vr

        edgesT = np.zeros((PD + 1, ECAP), dtype=E3NP)
        edgesT[PD, :] = 1.0
        edgesT[:PD, pos] = e8[eidx].T

        slot = node_slot[es]
        S_en = np.zeros((ECAP, P), dtype=F8NP)
        S_en[pos, slot] = 1.0
        S3 = S_en.reshape(NBLK, P, P)
        S_mat = np.ascontiguousarray(S3.transpose(1, 0, 2)).reshape(P, ECAP)
        ST_mat = np.ascontiguousarray(S3.transpose(2, 0, 1)).reshape(P, ECAP)
        del S_en, S3

        # q rows for this core's tiles, k-tile-split layout [128, TPC, 2, 128]
        qT = np.zeros((P, TPC, 2, P), dtype=np.float16)
        qr = np.zeros((TPC * P, QD), dtype=np.float16)
        own = core_of_tile[node_tile] == c
        rn = np.nonzero(own)[0]
        qpos = tloc_of_tile[node_tile[rn]] * P + node_slot[rn]
        qr[qpos] = np.asarray(q[rn], np.float32).astype(np.float16)
        qT[:] = qr.reshape(TPC, P, 2, P).transpose(3, 0, 2, 1)
        del qr

        cores.append(dict(kT=np.ascontiguousarray(kT.reshape(P, NBLK * 2 * P)),
                          vT=np.ascontiguousarray(vT.reshape(P, NBLK * 2 * P)),
                          edgesT=edgesT, S_mat=S_mat, ST_mat=ST_mat,
                          qT=np.ascontiguousarray(qT.reshape(P, TPC * 2 * P))))

    norm = D ** -0.5
    # weights in k-tile-split rhs layout [128kp, 2s, 128td]
    def w2(W, scale=1.0):
        Wf = np.asarray(W, np.float32) * scale      # [TD, QD]
        return np.ascontiguousarray(
            Wf.T.reshape(2, P, TD).transpose(1, 0, 2).astype(np.float16).reshape(P, 2 * TD))

    consts = dict(
        Wq2=w2(Wq, norm), Wk2=w2(Wk), Wv2=w2(Wv),
        WbT_aug=np.concatenate(
            [np.asarray(Wb, np.float32).T,
             np.asarray(bb, np.float32)[None, :]], axis=0).astype(np.float16),
        WoT=np.asarray(Wo, np.float32).T.astype(np.float16),
        bo_row=np.asarray(bo, np.float32)[None, :].astype(np.float16),
        identity=np.eye(P, dtype=np.float16),
        ones_col=np.ones((1, P), dtype=np.float16),
    )
    meta = dict(N=N, T=T, TPC=TPC, NBLK=NBLK, ECAP=ECAP,
                bpt=bpt.tolist(), blk_off=blk_off.tolist(),
                node_tile=node_tile, node_slot=node_slot, deg=deg,
                core_of_tile=core_of_tile, tloc_of_tile=tloc_of_tile)
    return cores, consts, meta


def build_program(meta, gather_batch=2):
    TPC = meta["TPC"]
    NBLK, ECAP = meta["NBLK"], meta["ECAP"]
    bpt, blk_off = meta["bpt"], meta["blk_off"]

    nc = bacc.Bacc("TRN2", target_bir_lowering=False, debug=False, num_devices=NC)
    dt = nc.dram_tensor
    t_kT = dt("kT", [P, NBLK * 2 * P], F16, kind="ExternalInput").ap()
    t_vT = dt("vT", [P, NBLK * 2 * P], F16, kind="ExternalInput").ap()
    t_eT = dt("edgesT", [PD + 1, ECAP], mybir.dt.float8e3, kind="ExternalInput").ap()
    t_S = dt("S_mat", [P, ECAP], F8, kind="ExternalInput").ap()
    t_ST = dt("ST_mat", [P, ECAP], F8, kind="ExternalInput").ap()
    t_qT = dt("qT", [P, TPC * 2 * P], F16, kind="ExternalInput").ap()
    t_Wq2 = dt("Wq2", [P, 2 * TD], F16, kind="ExternalInput").ap()
    t_Wk2 = dt("Wk2", [P, 2 * TD], F16, kind="ExternalInput").ap()
    t_Wv2 = dt("Wv2", [P, 2 * TD], F16, kind="ExternalInput").ap()
    t_Wb = dt("WbT_aug", [PD + 1, H], F16, kind="ExternalInput").ap()
    t_WoT = dt("WoT", [TD, QD], F16, kind="ExternalInput").ap()
    t_bo = dt("bo_row", [1, QD], F16, kind="ExternalInput").ap()
    t_id = dt("identity", [P, P], F16, kind="ExternalInput").ap()
    t_ones = dt("ones_col", [1, P], F16, kind="ExternalInput").ap()
    t_out = dt("o_out", [TPC * P, QD], F16, kind="ExternalOutput").ap()

    GB = gather_batch

    with tile.TileContext(nc) as tc:
        with (
            tc.tile_pool(name="const", bufs=1) as cpool,
            tc.tile_pool(name="gath", bufs=3) as gpool,
            tc.tile_pool(name="work", bufs=3) as wpool,
            tc.tile_pool(name="out", bufs=3) as opool,
            tc.tile_pool(name="psQ", bufs=2, space="PSUM") as psQ,
            tc.tile_pool(name="psKV", bufs=2, space="PSUM") as psKV,
            tc.tile_pool(name="psNE", bufs=2, space="PSUM") as psNE,
        ):
            # ---- constants ----
            c_Wq2 = cpool.tile([P, 2 * TD], F16); nc.sync.dma_start(out=c_Wq2[:], in_=t_Wq2)
            c_Wk2 = cpool.tile([P, 2 * TD], F16); nc.sync.dma_start(out=c_Wk2[:], in_=t_Wk2)
            c_Wv2 = cpool.tile([P, 2 * TD], F16); nc.sync.dma_start(out=c_Wv2[:], in_=t_Wv2)
            c_Wb = cpool.tile([PD + 1, H], F16); nc.sync.dma_start(out=c_Wb[:], in_=t_Wb)
            c_WoT = cpool.tile([TD, QD], F16); nc.sync.dma_start(out=c_WoT[:], in_=t_WoT)
            c_bo = cpool.tile([1, QD], F16); nc.sync.dma_start(out=c_bo[:], in_=t_bo)
            c_id = cpool.tile([P, P], F16); nc.sync.dma_start(out=c_id[:], in_=t_id)
            c_ones = cpool.tile([1, P], F16); nc.sync.dma_start(out=c_ones[:], in_=t_ones)
            qh_sb = cpool.tile([P, TPC * TD], F16)

            # ---- qh projection (f16, 2 accumulating matmuls) ----
            for g0 in range(0, TPC, 8):
                g1 = min(g0 + 8, TPC)
                qin = gpool.tile([P, 8 * 2 * P], F16, tag="qin")
                nc.sync.dma_start(out=qin[:, :(g1 - g0) * 2 * P],
                                  in_=t_qT[:, g0 * 2 * P:g1 * 2 * P])
                qv = qin[:, :(g1 - g0) * 2 * P].rearrange("p (t s e) -> p t s e", s=2, e=P)
                for t in range(g0, g1):
                    ps = psQ.tile([P, TD], F32, tag="q")
                    nc.tensor.matmul(out=ps[:], lhsT=qv[:, t - g0, 0, :],
                                     rhs=c_Wq2[:, 0:TD], start=True, stop=False)
                    nc.tensor.matmul(out=ps[:], lhsT=qv[:, t - g0, 1, :],
                                     rhs=c_Wq2[:, TD:2 * TD], start=False, stop=True)
                    nc.scalar.activation(out=qh_sb[:, t * TD:(t + 1) * TD],
                                         in_=ps[:], func=AF.Copy)

            # ---- main loop over tiles ----
            batches = []
            t0 = 0
            while t0 < TPC:
                t1 = min(t0 + GB, TPC)
                batches.append((t0, t1))
                t0 = t1
            BW = max(blk_off[t1] - blk_off[t0] for (t0, t1) in batches) * P
            MAXNB = max(bpt)

            for (b0, b1) in batches:
                e0, e1 = blk_off[b0] * P, blk_off[b1] * P
                ne = e1 - e0
                kT_sb = gpool.tile([P, 2 * BW], F16, tag="kT")
                nc.scalar.dma_start(out=kT_sb[:, :2 * ne], in_=t_kT[:, 2 * e0:2 * e1])
                vT_sb = gpool.tile([P, 2 * BW], F16, tag="vT")
                nc.scalar.dma_start(out=vT_sb[:, :2 * ne], in_=t_vT[:, 2 * e0:2 * e1])
                eT_sb = gpool.tile([PD + 1, BW], mybir.dt.float8e3, tag="eT")
                nc.sync.dma_start(out=eT_sb[:, :ne], in_=t_eT[:, e0:e1])
                S_sb = gpool.tile([P, BW], F8, tag="S")
                nc.sync.dma_start(out=S_sb[:, :ne], in_=t_S[:, e0:e1])
                ST_sb = gpool.tile([P, BW], F8, tag="ST")
                nc.sync.dma_start(out=ST_sb[:, :ne], in_=t_ST[:, e0:e1])
                kv_v = kT_sb[:, :2 * ne].rearrange("p (b s e) -> p b s e", s=2, e=P)
                vv_v = vT_sb[:, :2 * ne].rearrange("p (b s e) -> p b s e", s=2, e=P)

                for t in range(b0, b1):
                    nb = bpt[t]
                    go = blk_off[t] * P - e0
                    gb = go // P
                    ngrp = (nb + 3) // 4
                    # ne-bank layout (f32 cols): nd [0:136], eb [136:136+8nb],
                    # transpose scratch f16 [280:344], ps_o reuses [0:256]
                    ps_ne = psNE.tile([P, 344], F32, tag="ne")
                    wv = wpool.tile([P, MAXNB, 136], F16, tag="wv")
                    for g in range(ngrp):
                        gs = min(4, nb - g * 4)
                        ps_q4 = psQ.tile([P, 4, TD], F32, tag="q")
                        ps_kv4 = psKV.tile([P, 4, 2, TD], F32, tag="kv")
                        for j in range(gs):
                            b = g * 4 + j
                            nc.tensor.matmul(out=ps_kv4[:, j, 0, :],
                                             lhsT=kv_v[:, gb + b, 0, :],
                                             rhs=c_Wk2[:, 0:TD], start=True, stop=False)
                            nc.tensor.matmul(out=ps_kv4[:, j, 0, :],
                                             lhsT=kv_v[:, gb + b, 1, :],
                                             rhs=c_Wk2[:, TD:2 * TD], start=False, stop=True)
                            nc.tensor.matmul(out=ps_kv4[:, j, 1, :],
                                             lhsT=vv_v[:, gb + b, 0, :],
                                             rhs=c_Wv2[:, 0:TD], start=True, stop=False)
                            nc.tensor.matmul(out=ps_kv4[:, j, 1, :],
                                             lhsT=vv_v[:, gb + b, 1, :],
                                             rhs=c_Wv2[:, TD:2 * TD], start=False, stop=True)
                            nc.tensor.matmul(out=ps_q4[:, j, :],
                                             lhsT=ST_sb[:, go + b * P:go + (b + 1) * P],
                                             rhs=qh_sb[:, t * TD:(t + 1) * TD],
                                             start=True, stop=True)
                            nc.tensor.matmul(out=ps_ne[:, 136 + b * H:136 + (b + 1) * H],
                                             lhsT=eT_sb[:, go + b * P:go + (b + 1) * P],
                                             rhs=c_Wb[:], start=True, stop=True)
                        # ACT evacuates kh|vh for the group in one copy
                        khv4 = wpool.tile([P, 4, 2, TD], F16, tag="khv4")
                        nc.scalar.activation(out=khv4[:, 0:gs, :, :],
                                             in_=ps_kv4[:, 0:gs, :, :], func=AF.Copy)
                        prod = wpool.tile([P, 4, H, D], F16, tag="prod")
                        nc.vector.tensor_tensor(
                            out=prod[:, 0:gs, :, :],
                            in0=ps_q4[:, 0:gs, :].rearrange("p b (h d) -> p b h d", h=H),
                            in1=khv4[:, 0:gs, 0, :].rearrange("p b (h d) -> p b h d", h=H),
                            op=ALU.mult)
                        half = wpool.tile([P, 4, H, D // 2], F16, tag="half")
                        nc.vector.tensor_tensor(out=half[:, 0:gs, :, :],
                                                in0=prod[:, 0:gs, :, 0:D // 2],
                                                in1=prod[:, 0:gs, :, D // 2:D],
                                                op=ALU.add)
                        qk4 = wpool.tile([P, 4, H], F16, tag="qk4")
                        with nc.allow_low_precision(reason="f16 qk logits"):
                            nc.vector.reduce_sum(
                                out=qk4[:, 0:gs, :].rearrange("p b h -> p (b h)"),
                                in_=half[:, 0:gs, :, :].rearrange("p b h d -> p (b h) d"),
                                axis=mybir.AxisListType.X)
                        attn4 = wpool.tile([P, 4 * H], F32, tag="attn4")
                        nc.vector.tensor_tensor(
                            out=attn4[:, 0:gs * H],
                            in0=qk4[:, 0:gs, :].rearrange("p b h -> p (b h)"),
                            in1=ps_ne[:, 136 + g * 4 * H:136 + (g * 4 + gs) * H],
                            op=ALU.add)
                        # exp with broadcast-expanded output: w4x[e, b, h, d] = w[e, b, h]
                        w4x = wpool.tile([P, 4, H, D], F16, tag="w4x")
                        nc.scalar.activation(
                            out=w4x[:, 0:gs, :, :],
                            in_=attn4[:, 0:gs * H].rearrange(
                                "p (b h) -> p b h", h=H)[:, :, :, None].to_broadcast(
                                [P, gs, H, D]),
                            func=AF.Exp)
                        # weighted vh: both operands packed f16 (2x mode)
                        nc.vector.tensor_tensor(
                            out=wv[:, g * 4:g * 4 + gs, 0:TD],
                            in0=khv4[:, 0:gs, 1, :],
                            in1=w4x[:, 0:gs, :, :].rearrange("p b h d -> p b (h d)"),
                            op=ALU.mult)
                        nc.vector.tensor_copy(out=wv[:, g * 4:g * 4 + gs, TD:TD + H],
                                              in_=w4x[:, 0:gs, :, 0])
                    # scatter after ALL eb groups closed (same bank: the nd
                    # accumulation group must not interleave with eb starts)
                    for b in range(nb):
                        nc.tensor.matmul(out=ps_ne[:, 0:136],
                                         lhsT=S_sb[:, go + b * P:go + (b + 1) * P],
                                         rhs=wv[:, b, :],
                                         start=(b == 0), stop=(b == nb - 1))
                    # ---- finalize tile t inline (reusing the ne bank) ----
                    ndl = opool.tile([P, 136], F32, tag="ndl")
                    nc.vector.tensor_copy(out=ndl[:], in_=ps_ne[:, 0:136])
                    rden = opool.tile([P, H], F32, tag="rden")
                    nc.vector.tensor_scalar_add(out=rden[:], in0=ndl[:, TD:TD + H],
                                                scalar1=1e-30)
                    nc.vector.reciprocal(out=rden[:], in_=rden[:])
                    o_sb = opool.tile([P, TD], F16, tag="o_sb")
                    nc.vector.tensor_tensor(
                        out=o_sb[:].rearrange("p (h d) -> p h d", h=H),
                        in0=ndl[:, 0:TD].rearrange("p (h d) -> p h d", h=H),
                        in1=rden[:, :, None].to_broadcast([P, H, D]),
                        op=ALU.mult)
                    ps_oT = ps_ne[:, 280:344].bitcast(F16)      # [P, 128] f16
                    nc.tensor.transpose(out=ps_oT, in_=o_sb[:], identity=c_id[:])
                    oT_sb = opool.tile([P, P], F16, tag="oT_sb")
                    nc.vector.tensor_copy(out=oT_sb[:], in_=ps_oT)
                    ps_o = ps_ne[:, 0:256]
                    nc.tensor.matmul(out=ps_o, lhsT=oT_sb[:], rhs=c_WoT[:],
                                     start=True, stop=False)
                    nc.tensor.matmul(out=ps_o, lhsT=c_ones[:], rhs=c_bo[:],
                                     start=False, stop=True)
                    out_sb = opool.tile([P, QD], F16, tag="out_sb")
                    nc.vector.tensor_copy(out=out_sb[:], in_=ps_o)
                    nc.sync.dma_start(out=t_out[t * P:(t + 1) * P, :], in_=out_sb[:])

    nc.compile()
    return nc


_CACHE = {}
LAST_RUN = {}


def kernel(**inputs) -> np.ndarray:
    q = np.asarray(inputs["q"], np.float32)
    k = np.asarray(inputs["k"], np.float32)
    v = np.asarray(inputs["v"], np.float32)
    edges = np.asarray(inputs["edges"], np.float32)
    edge_index = np.asarray(inputs["edge_index"])
    Wq, Wk, Wv = inputs["Wq"], inputs["Wk"], inputs["Wv"]
    Wb, bb, Wo, bo = inputs["Wb"], inputs["bb"], inputs["Wo"], inputs["bo"]

    cores, consts, meta = prepare(q, k, v, edges, edge_index,
                                  Wq, Wk, Wv, Wb, bb, Wo, bo)
    N = meta["N"]
    TPC = meta["TPC"]

    key = (q.shape, edges.shape, meta["NBLK"])
    if key not in _CACHE:
        _CACHE[key] = build_program(meta)
    nc = _CACHE[key]

    in_maps = []
    for c in range(NC):
        m = dict(cores[c])
        m.update({kk: np.ascontiguousarray(vv) for kk, vv in consts.items()})
        in_maps.append(m)

    import os
    if os.environ.get("KERNEL_SIM"):
        from concourse.bass_interp import MultiCoreSim
        sim = MultiCoreSim(nc, num_cores=NC)
        for ci, core_sim in sim.cores.items():
            for name, arr in in_maps[ci].items():
                core_sim.tensor(name)[:] = arr
        sim.simulate(check_with_hw=False)
        results = [{"o_out": np.array(sim.cores[ci].tensor("o_out"))}
                   for ci in range(NC)]
    else:
        res = bass_utils.run_bass_kernel_spmd(nc, in_maps, core_ids=list(range(NC)))
        LAST_RUN["res"] = res
        results = res.results

    # assemble
    full = np.zeros((N, QD), np.float32)
    node_tile, node_slot = meta["node_tile"], meta["node_slot"]
    core_of_tile, tloc_of_tile = meta["core_of_tile"], meta["tloc_of_tile"]
    outs = [np.asarray(results[c]["o_out"], np.float32) for c in range(NC)]
    nt = node_tile[np.arange(N)]
    rows = tloc_of_tile[nt] * P + node_slot[np.arange(N)]
    for c in range(NC):
        m = core_of_tile[nt] == c
        full[m] = outs[c][rows[m]]
    zd = meta["deg"] == 0
    if zd.any():
        full[zd] = np.asarray(bo, np.float32)[None, :]
    return full


# revision 7
# speedup vs baseline: 1.2539x; 1.2539x over previous
"""GNN edge-softmax attention kernel for 8 Trainium2 NeuronCores — v2.

Strategy (8-way src-tile partition, zero collectives, zero dma_gather):
  - Host packs nodes into 128-slot tiles balanced by edge count (392 tiles),
    assigns 49 tiles to each core (LPT), and routes every edge to the core
    owning its src tile.  Per-core tile order is sorted by edge count so the
    shared block layout (bpt = max over cores) wastes <3% padding.
  - The host pre-gathers RAW k[dst] / v[dst] rows per edge (pure data
    movement; all arithmetic stays on device) as f16, k-tile-split for
    accumulating matmuls.  Each core streams these sequentially - no
    indirect DMA, no dma_gather descriptor-generation bottleneck.
  - Per 128-edge block on device: PE projects kh_e / vh_e (f16 matmuls),
    one-hot-gathers qh rows (ST matmul), computes edge-feature logits (eb);
    DVE forms qk = sum_d qh*kh, adds eb, ACT exponentiates; DVE weights vh;
    PE scatter-accumulates num|den into PSUM via one-hot S.
  - Finalize runs inline per tile (normalize + output projection); each core
    writes only its own 49 tiles.  Host re-permutes rows to node ids.
"""

import math
import sys

import numpy as np

sys.path.insert(0, "/opt/trn_rl_repo")

import concourse.bacc as bacc
import concourse.bass as bass
import concourse.mybir as mybir
import concourse.tile as tile
from concourse import bass_utils

F16 = mybir.dt.float16
F8 = mybir.dt.float8e4
F32 = mybir.dt.float32

H = 8            # heads
D = 16           # head dim
TD = H * D       # 128
QD = 256         # q/k/v feature dim
PD = 64          # edge pair feature dim
NC = 8           # cores
P = 128

AF = mybir.ActivationFunctionType
ALU = mybir.AluOpType
DR = mybir.MatmulPerfMode.DoubleRow


def _f8(x):
    import ml_dtypes
    return np.asarray(x, np.float32).astype(ml_dtypes.float8_e4m3)


def prepare(q, k, v, edges, edge_index, Wq, Wk, Wv, Wb, bb, Wo, bo):
    N = q.shape[0]
    T = NC * math.ceil(N / (NC * P))             # global tiles, mult of NC
    TPC = T // NC                                # tiles per core

    src = np.asarray(edge_index[:, 0], dtype=np.int64)
    dst = np.asarray(edge_index[:, 1], dtype=np.int64)
    deg = np.bincount(src, minlength=N)

    # --- greedy node->tile packing balanced by edge count ---
    order = np.argsort(-deg, kind="stable")
    tile_cnt = np.zeros(T, dtype=np.int64)
    tile_edges = np.zeros(T, dtype=np.int64)
    node_tile = np.zeros(N, dtype=np.int32)
    node_slot = np.zeros(N, dtype=np.int32)
    import heapq
    heap = [(0, t) for t in range(T)]
    heapq.heapify(heap)
    for n in order:
        while True:
            e_cnt, t = heapq.heappop(heap)
            if tile_cnt[t] < P:
                break
        node_tile[n] = t
        node_slot[n] = tile_cnt[t]
        tile_cnt[t] += 1
        tile_edges[t] += deg[n]
        if tile_cnt[t] < P:
            heapq.heappush(heap, (tile_edges[t], t))

    # --- tile -> core (LPT) then per-core order by count desc ---
    t_order = np.argsort(-tile_edges, kind="stable")
    core_load = [(0, c) for c in range(NC)]
    heapq.heapify(core_load)
    core_tiles = [[] for _ in range(NC)]
    for t in t_order:
        load, c = heapq.heappop(core_load)
        core_tiles[c].append(t)
        heapq.heappush(core_load, (load + int(tile_edges[t]), c))
    # per-core local order: by edge count desc (aligns heavy tiles at the
    # same tloc across cores so the shared bpt is tight)
    for c in range(NC):
        core_tiles[c].sort(key=lambda t: -int(tile_edges[t]))
    tile_of = np.zeros((NC, TPC), dtype=np.int64)    # (core, tloc) -> tile
    core_of_tile = np.zeros(T, dtype=np.int64)
    tloc_of_tile = np.zeros(T, dtype=np.int64)
    for c in range(NC):
        for i, t in enumerate(core_tiles[c]):
            tile_of[c, i] = t
            core_of_tile[t] = c
            tloc_of_tile[t] = i

    cnt = np.zeros((NC, TPC), dtype=np.int64)
    for c in range(NC):
        cnt[c] = tile_edges[tile_of[c]]
    bpt = np.maximum(1, np.ceil(cnt.max(axis=0) / P).astype(np.int64))  # [TPC]
    blk_off = np.concatenate([[0], np.cumsum(bpt)])
    NBLK = int(blk_off[-1])
    ECAP = NBLK * P

    # --- per-core edge arrays ---
    import ml_dtypes
    F8NP = ml_dtypes.float8_e4m3
    E3NP = ml_dtypes.float8_e3m4
    k8 = np.asarray(k, np.float32).astype(np.float16)   # [N, QD]
    v8 = np.asarray(v, np.float32).astype(np.float16)
    e8 = np.asarray(edges, np.float32).astype(E3NP)     # [E, PD]

    edge_core = core_of_tile[node_tile[src]]
    edge_tloc = tloc_of_tile[node_tile[src]]

    cores = []
    for c in range(NC):
        mask = edge_core == c
        es, ed, et = src[mask], dst[mask], edge_tloc[mask]
        ordr = np.argsort(et, kind="stable")
        es, ed, et = es[ordr], ed[ordr], et[ordr]
        eidx = np.nonzero(mask)[0][ordr]
        # position per edge: tile t's edges fill from blk_off[t]*P
        pos = np.zeros(len(es), dtype=np.int64)
        start = 0
        for t in range(TPC):
            ccc = int(cnt[c, t])
            pos[start:start + ccc] = blk_off[t] * P + np.arange(ccc)
            start += ccc

        # per-edge raw k/v rows, k-tile-split layout [128, NBLK, 2, 128]
        kT = np.zeros((P, NBLK, 2, P), dtype=np.float16)
        vT = np.zeros((P, NBLK, 2, P), dtype=np.float16)
        kr = np.zeros((ECAP, QD), dtype=np.float16)
        vr = np.zeros((ECAP, QD), dtype=np.float16)
        kr[pos] = k8[ed]
        vr[pos] = v8[ed]
        # [ECAP, 256] -> [NBLK, 128e, 2, 128kp] -> [kp, b, s, e]
        kT[:] = kr.reshape(NBLK, P, 2, P).transpose(3, 0, 2, 1)
        vT[:] = vr.reshape(NBLK, P, 2, P).transpose(3, 0, 2, 1)
        del kr, vr

        edgesT = np.zeros((PD + 1, ECAP), dtype=E3NP)
        edgesT[PD, :] = 1.0
        edgesT[:PD, pos] = e8[eidx].T

        slot = node_slot[es]
        S_en = np.zeros((ECAP, P), dtype=F8NP)
        S_en[pos, slot] = 1.0
        S3 = S_en.reshape(NBLK, P, P)
        S_mat = np.ascontiguousarray(S3.transpose(1, 0, 2)).reshape(P, ECAP)
        ST_mat = np.ascontiguousarray(S3.transpose(2, 0, 1)).reshape(P, ECAP)
        del S_en, S3

        # q rows for this core's tiles, k-tile-split layout [128, TPC, 2, 128]
        qT = np.zeros((P, TPC, 2, P), dtype=np.float16)
        qr = np.zeros((TPC * P, QD), dtype=np.float16)
        own = core_of_tile[node_tile] == c
        rn = np.nonzero(own)[0]
        qpos = tloc_of_tile[node_tile[rn]] * P + node_slot[rn]
        qr[qpos] = np.asarray(q[rn], np.float32).astype(np.float16)
        qT[:] = qr.reshape(TPC, P, 2, P).transpose(3, 0, 2, 1)
        del qr

        cores.append(dict(kT=np.ascontiguousarray(kT.reshape(P, NBLK * 2 * P)),
                          vT=np.ascontiguousarray(vT.reshape(P, NBLK * 2 * P)),
                          edgesT=edgesT, S_mat=S_mat, ST_mat=ST_mat,
                          qT=np.ascontiguousarray(qT.reshape(P, TPC * 2 * P))))

    norm = D ** -0.5
    # weights in k-tile-split rhs layout [128kp, 2s, 128td]
    def w2(W, scale=1.0):
        Wf = np.asarray(W, np.float32) * scale      # [TD, QD]
        return np.ascontiguousarray(
            Wf.T.reshape(2, P, TD).transpose(1, 0, 2).astype(np.float16).reshape(P, 2 * TD))

    consts = dict(
        Wq2=w2(Wq, norm), Wk2=w2(Wk), Wv2=w2(Wv),
        WbT_aug=np.concatenate(
            [np.asarray(Wb, np.float32).T,
             np.asarray(bb, np.float32)[None, :]], axis=0).astype(np.float16),
        WoT=np.asarray(Wo, np.float32).T.astype(np.float16),
        bo_row=np.asarray(bo, np.float32)[None, :].astype(np.float16),
        identity=np.eye(P, dtype=np.float16),
        ones_col=np.ones((1, P), dtype=np.float16),
    )
    meta = dict(N=N, T=T, TPC=TPC, NBLK=NBLK, ECAP=ECAP,
                bpt=bpt.tolist(), blk_off=blk_off.tolist(),
                node_tile=node_tile, node_slot=node_slot, deg=deg,
                core_of_tile=core_of_tile, tloc_of_tile=tloc_of_tile)
    return cores, consts, meta


def build_program(meta, gather_batch=2):
    TPC = meta["TPC"]
    NBLK, ECAP = meta["NBLK"], meta["ECAP"]
    bpt, blk_off = meta["bpt"], meta["blk_off"]

    nc = bacc.Bacc("TRN2", target_bir_lowering=False, debug=False, num_devices=NC)
    dt = nc.dram_tensor
    t_kT = dt("kT", [P, NBLK * 2 * P], F16, kind="ExternalInput").ap()
    t_vT = dt("vT", [P, NBLK * 2 * P], F16, kind="ExternalInput").ap()
    t_eT = dt("edgesT", [PD + 1, ECAP], mybir.dt.float8e3, kind="ExternalInput").ap()
    t_S = dt("S_mat", [P, ECAP], F8, kind="ExternalInput").ap()
    t_ST = dt("ST_mat", [P, ECAP], F8, kind="ExternalInput").ap()
    t_qT = dt("qT", [P, TPC * 2 * P], F16, kind="ExternalInput").ap()
    t_Wq2 = dt("Wq2", [P, 2 * TD], F16, kind="ExternalInput").ap()
    t_Wk2 = dt("Wk2", [P, 2 * TD], F16, kind="ExternalInput").ap()
    t_Wv2 = dt("Wv2", [P, 2 * TD], F16, kind="ExternalInput").ap()
    t_Wb = dt("WbT_aug", [PD + 1, H], F16, kind="ExternalInput").ap()
    t_WoT = dt("WoT", [TD, QD], F16, kind="ExternalInput").ap()
    t_bo = dt("bo_row", [1, QD], F16, kind="ExternalInput").ap()
    t_id = dt("identity", [P, P], F16, kind="ExternalInput").ap()
    t_ones = dt("ones_col", [1, P], F16, kind="ExternalInput").ap()
    t_out = dt("o_out", [TPC * P, QD], F16, kind="ExternalOutput").ap()

    GB = gather_batch

    with tile.TileContext(nc) as tc:
        with (
            tc.tile_pool(name="const", bufs=1) as cpool,
            tc.tile_pool(name="gath", bufs=3) as gpool,
            tc.tile_pool(name="work", bufs=3) as wpool,
            tc.tile_pool(name="out", bufs=3) as opool,
            tc.tile_pool(name="psQ", bufs=2, space="PSUM") as psQ,
            tc.tile_pool(name="psKV", bufs=2, space="PSUM") as psKV,
            tc.tile_pool(name="psNE", bufs=2, space="PSUM") as psNE,
        ):
            # ---- constants ----
            c_Wq2 = cpool.tile([P, 2 * TD], F16); nc.sync.dma_start(out=c_Wq2[:], in_=t_Wq2)
            c_Wk2 = cpool.tile([P, 2 * TD], F16); nc.sync.dma_start(out=c_Wk2[:], in_=t_Wk2)
            c_Wv2 = cpool.tile([P, 2 * TD], F16); nc.sync.dma_start(out=c_Wv2[:], in_=t_Wv2)
            c_Wb = cpool.tile([PD + 1, H], F16); nc.sync.dma_start(out=c_Wb[:], in_=t_Wb)
            c_WoT = cpool.tile([TD, QD], F16); nc.sync.dma_start(out=c_WoT[:], in_=t_WoT)
            c_bo = cpool.tile([1, QD], F16); nc.sync.dma_start(out=c_bo[:], in_=t_bo)
            c_id = cpool.tile([P, P], F16); nc.sync.dma_start(out=c_id[:], in_=t_id)
            c_ones = cpool.tile([1, P], F16); nc.sync.dma_start(out=c_ones[:], in_=t_ones)
            qh_sb = cpool.tile([P, TPC * TD], F16)

            # ---- qh projection (f16, 2 accumulating matmuls) ----
            for g0 in range(0, TPC, 8):
                g1 = min(g0 + 8, TPC)
                qin = gpool.tile([P, 8 * 2 * P], F16, tag="qin")
                nc.sync.dma_start(out=qin[:, :(g1 - g0) * 2 * P],
                                  in_=t_qT[:, g0 * 2 * P:g1 * 2 * P])
                qv = qin[:, :(g1 - g0) * 2 * P].rearrange("p (t s e) -> p t s e", s=2, e=P)
                for t in range(g0, g1):
                    ps = psQ.tile([P, TD], F32, tag="q")
                    nc.tensor.matmul(out=ps[:], lhsT=qv[:, t - g0, 0, :],
                                     rhs=c_Wq2[:, 0:TD], start=True, stop=False)
                    nc.tensor.matmul(out=ps[:], lhsT=qv[:, t - g0, 1, :],
                                     rhs=c_Wq2[:, TD:2 * TD], start=False, stop=True)
                    nc.scalar.activation(out=qh_sb[:, t * TD:(t + 1) * TD],
                                         in_=ps[:], func=AF.Copy)

            # ---- main loop over tiles ----
            batches = []
            t0 = 0
            while t0 < TPC:
                t1 = min(t0 + GB, TPC)
                batches.append((t0, t1))
                t0 = t1
            BW = max(blk_off[t1] - blk_off[t0] for (t0, t1) in batches) * P
            MAXNB = max(bpt)

            for (b0, b1) in batches:
                e0, e1 = blk_off[b0] * P, blk_off[b1] * P
                ne = e1 - e0
                kT_sb = gpool.tile([P, 2 * BW], F16, tag="kT")
                nc.sync.dma_start(out=kT_sb[:, :2 * ne], in_=t_kT[:, 2 * e0:2 * e1])
                vT_sb = gpool.tile([P, 2 * BW], F16, tag="vT")
                nc.sync.dma_start(out=vT_sb[:, :2 * ne], in_=t_vT[:, 2 * e0:2 * e1])
                eT_sb = gpool.tile([PD + 1, BW], mybir.dt.float8e3, tag="eT")
                nc.sync.dma_start(out=eT_sb[:, :ne], in_=t_eT[:, e0:e1])
                S_sb = gpool.tile([P, BW], F8, tag="S")
                nc.sync.dma_start(out=S_sb[:, :ne], in_=t_S[:, e0:e1])
                ST_sb = gpool.tile([P, BW], F8, tag="ST")
                nc.sync.dma_start(out=ST_sb[:, :ne], in_=t_ST[:, e0:e1])
                kv_v = kT_sb[:, :2 * ne].rearrange("p (b s e) -> p b s e", s=2, e=P)
                vv_v = vT_sb[:, :2 * ne].rearrange("p (b s e) -> p b s e", s=2, e=P)

                for t in range(b0, b1):
                    nb = bpt[t]
                    go = blk_off[t] * P - e0
                    gb = go // P
                    ngrp = (nb + 3) // 4
                    # ne-bank layout (f32 cols): nd [0:136], eb [136:136+8nb],
                    # transpose scratch f16 [280:344], ps_o reuses [0:256]
                    ps_ne = psNE.tile([P, 344], F32, tag="ne")
                    wv = wpool.tile([P, MAXNB, 136], F16, tag="wv")
                    for g in range(ngrp):
                        gs = min(4, nb - g * 4)
                        ps_q4 = psQ.tile([P, 4, TD], F32, tag="q")
                        ps_kv4 = psKV.tile([P, 4, 2, TD], F32, tag="kv")
                        for j in range(gs):
                            b = g * 4 + j
                            nc.tensor.matmul(out=ps_kv4[:, j, 0, :],
                                             lhsT=kv_v[:, gb + b, 0, :],
                                             rhs=c_Wk2[:, 0:TD], start=True, stop=False)
                            nc.tensor.matmul(out=ps_kv4[:, j, 0, :],
                                             lhsT=kv_v[:, gb + b, 1, :],
                                             rhs=c_Wk2[:, TD:2 * TD], start=False, stop=True)
                            nc.tensor.matmul(out=ps_kv4[:, j, 1, :],
                                             lhsT=vv_v[:, gb + b, 0, :],
                                             rhs=c_Wv2[:, 0:TD], start=True, stop=False)
                            nc.tensor.matmul(out=ps_kv4[:, j, 1, :],
                                             lhsT=vv_v[:, gb + b, 1, :],
                                             rhs=c_Wv2[:, TD:2 * TD], start=False, stop=True)
                            nc.tensor.matmul(out=ps_q4[:, j, :],
                                             lhsT=ST_sb[:, go + b * P:go + (b + 1) * P],
                                             rhs=qh_sb[:, t * TD:(t + 1) * TD],
                                             start=True, stop=True)
                            nc.tensor.matmul(out=ps_ne[:, 136 + b * H:136 + (b + 1) * H],
                                             lhsT=eT_sb[:, go + b * P:go + (b + 1) * P],
                                             rhs=c_Wb[:], start=True, stop=True)
                        # ACT evacuates kh|vh for the group in one copy
                        khv4 = wpool.tile([P, 4, 2, TD], F16, tag="khv4")
                        nc.scalar.activation(out=khv4[:, 0:gs, :, :],
                                             in_=ps_kv4[:, 0:gs, :, :], func=AF.Copy)
                        prod = wpool.tile([P, 4, H, D], F16, tag="prod")
                        nc.vector.tensor_tensor(
                            out=prod[:, 0:gs, :, :],
                            in0=ps_q4[:, 0:gs, :].rearrange("p b (h d) -> p b h d", h=H),
                            in1=khv4[:, 0:gs, 0, :].rearrange("p b (h d) -> p b h d", h=H),
                            op=ALU.mult)
                        qk4 = wpool.tile([P, 4, H], F16, tag="qk4")
                        with nc.allow_low_precision(reason="f16 qk logits"):
                            nc.vector.reduce_sum(
                                out=qk4[:, 0:gs, :].rearrange("p b h -> p (b h)"),
                                in_=prod[:, 0:gs, :, :].rearrange("p b h d -> p (b h) d"),
                                axis=mybir.AxisListType.X)
                        attn4 = wpool.tile([P, 4 * H], F32, tag="attn4")
                        nc.vector.tensor_tensor(
                            out=attn4[:, 0:gs * H],
                            in0=qk4[:, 0:gs, :].rearrange("p b h -> p (b h)"),
                            in1=ps_ne[:, 136 + g * 4 * H:136 + (g * 4 + gs) * H],
                            op=ALU.add)
                        # exp with broadcast-expanded output: w4x[e, b, h, d] = w[e, b, h]
                        w4x = wpool.tile([P, 4, H, D], F16, tag="w4x")
                        nc.scalar.activation(
                            out=w4x[:, 0:gs, :, :],
                            in_=attn4[:, 0:gs * H].rearrange(
                                "p (b h) -> p b h", h=H)[:, :, :, None].to_broadcast(
                                [P, gs, H, D]),
                            func=AF.Exp)
                        # weighted vh: both operands packed f16 (2x mode)
                        nc.vector.tensor_tensor(
                            out=wv[:, g * 4:g * 4 + gs, 0:TD],
                            in0=khv4[:, 0:gs, 1, :],
                            in1=w4x[:, 0:gs, :, :].rearrange("p b h d -> p b (h d)"),
                            op=ALU.mult)
                        nc.vector.tensor_copy(out=wv[:, g * 4:g * 4 + gs, TD:TD + H],
                                              in_=w4x[:, 0:gs, :, 0])
                    # scatter after ALL eb groups closed (same bank: the nd
                    # accumulation group must not interleave with eb starts)
                    for b in range(nb):
                        nc.tensor.matmul(out=ps_ne[:, 0:136],
                                         lhsT=S_sb[:, go + b * P:go + (b + 1) * P],
                                         rhs=wv[:, b, :],
                                         start=(b == 0), stop=(b == nb - 1))
                    # ---- finalize tile t inline (reusing the ne bank) ----
                    ndl = opool.tile([P, 136], F32, tag="ndl")
                    nc.vector.tensor_copy(out=ndl[:], in_=ps_ne[:, 0:136])
                    rden = opool.tile([P, H], F32, tag="rden")
                    nc.vector.tensor_scalar_add(out=rden[:], in0=ndl[:, TD:TD + H],
                                                scalar1=1e-30)
                    nc.vector.reciprocal(out=rden[:], in_=rden[:])
                    o_sb = opool.tile([P, TD], F16, tag="o_sb")
                    nc.vector.tensor_tensor(
                        out=o_sb[:].rearrange("p (h d) -> p h d", h=H),
                        in0=ndl[:, 0:TD].rearrange("p (h d) -> p h d", h=H),
                        in1=rden[:, :, None].to_broadcast([P, H, D]),
                        op=ALU.mult)
                    ps_oT = ps_ne[:, 280:344].bitcast(F16)      # [P, 128] f16
                    nc.tensor.transpose(out=ps_oT, in_=o_sb[:], identity=c_id[:])
                    oT_sb = opool.tile([P, P], F16, tag="oT_sb")
                    nc.vector.tensor_copy(out=oT_sb[:], in_=ps_oT)
                    ps_o = ps_ne[:, 0:256]
                    nc.tensor.matmul(out=ps_o, lhsT=oT_sb[:], rhs=c_WoT[:],
                                     start=True, stop=False)
                    nc.tensor.matmul(out=ps_o, lhsT=c_ones[:], rhs=c_bo[:],
                                     start=False, stop=True)
                    out_sb = opool.tile([P, QD], F16, tag="out_sb")
                    nc.vector.tensor_copy(out=out_sb[:], in_=ps_o)
                    nc.sync.dma_start(out=t_out[t * P:(t + 1) * P, :], in_=out_sb[:])

    nc.compile()
    return nc


_CACHE = {}
LAST_RUN = {}


def kernel(**inputs) -> np.ndarray:
    q = np.asarray(inputs["q"], np.float32)
    k = np.asarray(inputs["k"], np.float32)
    v = np.asarray(inputs["v"], np.float32)
    edges = np.asarray(inputs["edges"], np.float32)
    edge_index = np.asarray(inputs["edge_index"])
    Wq, Wk, Wv = inputs["Wq"], inputs["Wk"], inputs["Wv"]
    Wb, bb, Wo, bo = inputs["Wb"], inputs["bb"], inputs["Wo"], inputs["bo"]

    cores, consts, meta = prepare(q, k, v, edges, edge_index,
                                  Wq, Wk, Wv, Wb, bb, Wo, bo)
    N = meta["N"]
    TPC = meta["TPC"]

    key = (q.shape, edges.shape, meta["NBLK"])
    if key not in _CACHE:
        _CACHE[key] = build_program(meta)
    nc = _CACHE[key]

    in_maps = []
    for c in range(NC):
        m = dict(cores[c])
        m.update({kk: np.ascontiguousarray(vv) for kk, vv in consts.items()})
        in_maps.append(m)

    import os
    if os.environ.get("KERNEL_SIM"):
        from concourse.bass_interp import MultiCoreSim
        sim = MultiCoreSim(nc, num_cores=NC)
        for ci, core_sim in sim.cores.items():
            for name, arr in in_maps[ci].items():
                core_sim.tensor(name)[:] = arr
        sim.simulate(check_with_hw=False)
        results = [{"o_out": np.array(sim.cores[ci].tensor("o_out"))}
                   for ci in range(NC)]
    else:
        res = bass_utils.run_bass_kernel_spmd(nc, in_maps, core_ids=list(range(NC)))
        LAST_RUN["res"] = res
        results = res.results

    # assemble
    full = np.zeros((N, QD), np.float32)
    node_tile, node_slot = meta["node_tile"], meta["node_slot"]
    core_of_tile, tloc_of_tile = meta["core_of_tile"], meta["tloc_of_tile"]
    outs = [np.asarray(results[c]["o_out"], np.float32) for c in range(NC)]
    nt = node_tile[np.arange(N)]
    rows = tloc_of_tile[nt] * P + node_slot[np.arange(N)]
    for c in range(NC):
        m = core_of_tile[nt] == c
        full[m] = outs[c][rows[m]]
    zd = meta["deg"] == 0
    if zd.any():
        full[zd] = np.asarray(bo, np.float32)[None, :]
    return full
